# revision 9
# baseline (speedup 1.0000x reference)
"""Bass/Trainium2 kernel for the BayesTensorRing embedding-lookup problem.

out[i] = trace( prod_d (core_d[idx_d] * diag(lam_d)) ),  N=500k rows, 8 cores.

Strategy:
  * Host (one-time weight repacking, independent of the index stream):
    fold each lambda into its core, then build two pair-product tables
       T1[(j0,j1)][x,z] = sum_y A0[j0,x,y] A1[j1,y,z]     (A0A1)
       T2[(j2,j3)][x,z] = sum_w A2[j2,z,w] A3[j3,w,x]     ((A2A3)^T layout)
    each [40000, 256] fp16.  Then out[i] = <T1[p_i], T2[q_i]> elementwise dot,
    p = i0*200+i1, q = i2*200+i3.
  * Device (all per-row work): shard N over 8 NeuronCores. Rows are bucketed
    by (p>=32768, q>=32768) so each bulk dma_gather chunk addresses one table
    half with int16 indices. Per 2048-row chunk: two dma_gathers (512B rows)
    + one DVE multiply + one DVE reduce -> 2048 fp32 outputs.
"""

import os
import sys

import numpy as np

sys.path.insert(0, "/opt/trn_rl_repo")

from concourse import bacc, bass, mybir
import concourse.tile as tile
from concourse.bass_utils import run_bass_kernel_spmd

NCORES = 8
DIM = 200
R = 16
RR = R * R  # 256
NTAB = DIM * DIM  # 40000
HALF = 32768
N = 500_000
P = 128
CHUNK = 1024  # rows per dma_gather (2048 fails on HW)
SUB = CHUNK // P  # 16 sub-tiles of 128 rows per chunk


def _tables(core0, core1, core2, core3, lam0, lam1, lam2, lam3):
    A0 = (core0 * lam0[None, None, :]).astype(np.float32)
    A1 = (core1 * lam1[None, None, :]).astype(np.float32)
    A2 = (core2 * lam2[None, None, :]).astype(np.float32)
    A3 = (core3 * lam3[None, None, :]).astype(np.float32)
    # T1[(j0,j1)][x,z] = sum_y A0[j0,x,y] A1[j1,y,z]
    M1 = A0.reshape(DIM * R, R) @ np.ascontiguousarray(A1.transpose(1, 0, 2)).reshape(
        R, DIM * R
    )  # [(j0 x), (j1 z)]
    T1 = np.ascontiguousarray(
        M1.reshape(DIM, R, DIM, R).transpose(0, 2, 1, 3)
    ).reshape(NTAB, RR)
    # T2[(j2,j3)][x,z] = sum_w A2[j2,z,w] A3[j3,w,x]
    M2 = A2.reshape(DIM * R, R) @ np.ascontiguousarray(A3.transpose(1, 0, 2)).reshape(
        R, DIM * R
    )  # [(j2 z), (j3 x)]
    T2 = np.ascontiguousarray(
        M2.reshape(DIM, R, DIM, R).transpose(0, 2, 3, 1)
    ).reshape(NTAB, RR)
    return T1.astype(np.float16), T2.astype(np.float16)


def _wrap16(a):
    """Pack a [n*128] idx list into dma_gather layout [128, n*8]:
    idx i -> [i % 16, i // 16], replicated over the 8 partition groups."""
    w = np.ascontiguousarray(a.reshape(-1, 16).T)  # [16, len/16]
    return np.ascontiguousarray(np.tile(w, (8, 1)))  # [128, len/16]


def build_program(flavors):
    """flavors: tuple of (p_hi, q_hi) per chunk — static table-half selection."""
    nch = len(flavors)
    nc = bacc.Bacc("TRN2", target_bir_lowering=False, num_swdge_queues=4)
    f16 = mybir.dt.float16
    f32 = mybir.dt.float32
    i16 = mybir.dt.int16

    t1 = nc.dram_tensor("t1", [NTAB, RR], f16, kind="ExternalInput")
    t2 = nc.dram_tensor("t2", [NTAB, RR], f16, kind="ExternalInput")
    # per chunk: CHUNK idxs wrapped to [128, CHUNK//16]
    pidx = nc.dram_tensor("pidx", [P, nch * SUB * 8], i16, kind="ExternalInput")
    qidx = nc.dram_tensor("qidx", [P, nch * SUB * 8], i16, kind="ExternalInput")
    out = nc.dram_tensor("out", [P, nch * SUB], f32, kind="ExternalOutput")

    IW = SUB * 8  # idx words per chunk per partition (CHUNK/16)

    with tile.TileContext(nc) as tc:
        with (
            tc.tile_pool(name="idx", bufs=8) as idx_pool,
            tc.tile_pool(name="g1", bufs=4) as g1_pool,
            tc.tile_pool(name="g2", bufs=4) as g2_pool,
            tc.tile_pool(name="prod", bufs=4) as prod_pool,
            tc.tile_pool(name="res", bufs=1) as res_pool,
        ):
            out_sb = res_pool.tile([P, nch * SUB], f32)

            for c, (p_hi, q_hi) in enumerate(flavors):
                src1 = t1[HALF:NTAB, :] if p_hi else t1[0:HALF, :]
                src2 = t2[HALF:NTAB, :] if q_hi else t2[0:HALF, :]
                p_sb = idx_pool.tile([P, IW], i16, tag="p")
                q_sb = idx_pool.tile([P, IW], i16, tag="q")
                nc.sync.dma_start(out=p_sb[:], in_=pidx[:, c * IW : (c + 1) * IW])
                nc.sync.dma_start(out=q_sb[:], in_=qidx[:, c * IW : (c + 1) * IW])
                g1 = g1_pool.tile([P, SUB, RR], f16)
                g2 = g2_pool.tile([P, SUB, RR], f16)
                nc.gpsimd.dma_gather(
                    g1[:], src1, p_sb[:], CHUNK, CHUNK, RR,
                    queue_num=(2 * c) % 4,
                )
                nc.gpsimd.dma_gather(
                    g2[:], src2, q_sb[:], CHUNK, CHUNK, RR,
                    queue_num=(2 * c + 1) % 4,
                )
                prod = prod_pool.tile([P, SUB, RR], f16)
                nc.vector.tensor_tensor(
                    out=prod[:], in0=g1[:], in1=g2[:], op=mybir.AluOpType.mult
                )
                nc.vector.tensor_reduce(
                    out=out_sb[:, c * SUB : (c + 1) * SUB],
                    in_=prod[:],
                    axis=mybir.AxisListType.X,
                    op=mybir.AluOpType.add,
                )
            nc.sync.dma_start(out=out[:], in_=out_sb[:])
    nc.compile()
    return nc


_PROG_CACHE = {}


def _get_program(flavors):
    key = tuple(flavors)
    if key not in _PROG_CACHE:
        _PROG_CACHE[key] = build_program(key)
    return _PROG_CACHE[key]


def plan(index):
    """Bucket rows per core, build a shared chunk grid + per-core idx arrays."""
    idx = np.asarray(index).astype(np.int64)
    p_all = (idx[:, 0] * DIM + idx[:, 1]).astype(np.int32)
    q_all = (idx[:, 2] * DIM + idx[:, 3]).astype(np.int32)
    rows = N // NCORES
    per_core = []
    counts = np.zeros((NCORES, 4), np.int64)
    for c in range(NCORES):
        sl = slice(c * rows, (c + 1) * rows)
        p, q = p_all[sl], q_all[sl]
        b = (p >= HALF) * 2 + (q >= HALF)
        ids = [np.where(b == bb)[0] for bb in range(4)]
        counts[c] = [len(x) for x in ids]
        per_core.append((p, q, ids))
    nch_b = [max(1, -(-int(counts[:, bb].max()) // CHUNK)) for bb in range(4)]
    flavors = []
    for bb in range(4):
        flavors += [(bb >> 1 & 1, bb & 1)] * nch_b[bb]
    return per_core, nch_b, tuple(flavors)


def make_in_maps(per_core, nch_b, T1, T2):
    in_maps, metas = [], []
    for p, q, ids in per_core:
        p16s, q16s, meta = [], [], []
        for bb in range(4):
            cap = nch_b[bb] * CHUNK
            sel = ids[bb]
            pd = np.full(cap, HALF if (bb >> 1 & 1) else 0, np.int32)
            qd = np.full(cap, HALF if (bb & 1) else 0, np.int32)
            pd[: len(sel)] = p[sel]
            qd[: len(sel)] = q[sel]
            p16s.append((pd - (HALF if (bb >> 1 & 1) else 0)).astype(np.int16))
            q16s.append((qd - (HALF if (bb & 1) else 0)).astype(np.int16))
            meta.append(sel)
        p16 = np.concatenate(p16s)
        q16 = np.concatenate(q16s)
        # wrap each chunk independently and lay side by side on the free dim
        nch = len(p16) // CHUNK
        pw = np.concatenate(
            [_wrap16(p16[i * CHUNK : (i + 1) * CHUNK]) for i in range(nch)], axis=1
        )
        qw = np.concatenate(
            [_wrap16(q16[i * CHUNK : (i + 1) * CHUNK]) for i in range(nch)], axis=1
        )
        in_maps.append({"t1": T1, "t2": T2, "pidx": pw, "qidx": qw})
        metas.append(meta)
    return in_maps, metas


def unpack(results, metas, nch_b):
    rows = N // NCORES
    outs = []
    for c in range(NCORES):
        o = results[c]["out"]  # [128, nch*SUB]; slot s -> [s%128, s//128]
        flat = np.asarray(o).T.reshape(-1)  # slot-ordered
        full = np.empty(rows, np.float32)
        off = 0
        for bb in range(4):
            cap = nch_b[bb] * CHUNK
            sel = metas[c][bb]
            full[sel] = flat[off : off + len(sel)]
            off += cap
        outs.append(full)
    return np.concatenate(outs)


def kernel(index, core0, core1, core2, core3, lam0, lam1, lam2, lam3, _trace=False):
    T1, T2 = _tables(
        np.asarray(core0), np.asarray(core1), np.asarray(core2), np.asarray(core3),
        np.asarray(lam0), np.asarray(lam1), np.asarray(lam2), np.asarray(lam3),
    )
    per_core, nch_b, flavors = plan(index)
    nc = _get_program(flavors)
    in_maps, metas = make_in_maps(per_core, nch_b, T1, T2)
    res = run_bass_kernel_spmd(
        nc, in_maps, core_ids=list(range(NCORES)), trace=_trace
    )
    full = unpack(res.results, metas, nch_b).astype(np.float32)
    if _trace:
        return full, res
    return full


# revision 10
# speedup vs baseline: 1.0775x; 1.0775x over previous
"""Bass/Trainium2 kernel for the BayesTensorRing embedding-lookup problem.

out[i] = trace( prod_d (core_d[idx_d] * diag(lam_d)) ),  N=500k rows, 8 cores.

Strategy:
  * Host (one-time weight repacking, independent of the index stream):
    fold each lambda into its core, then build two pair-product tables
       T1[(j0,j1)][x,z] = sum_y A0[j0,x,y] A1[j1,y,z]     (A0A1)
       T2[(j2,j3)][x,z] = sum_w A2[j2,z,w] A3[j3,w,x]     ((A2A3)^T layout)
    each [40000, 256] fp16.  Then out[i] = <T1[p_i], T2[q_i]> elementwise dot,
    p = i0*200+i1, q = i2*200+i3.
  * Device (all per-row work): shard N over 8 NeuronCores. Rows are bucketed
    by (p>=32768, q>=32768) so each bulk dma_gather chunk addresses one table
    half with int16 indices. Per 2048-row chunk: two dma_gathers (512B rows)
    + one DVE multiply + one DVE reduce -> 2048 fp32 outputs.
"""

import os
import sys

import numpy as np

sys.path.insert(0, "/opt/trn_rl_repo")

from concourse import bacc, bass, mybir
import concourse.tile as tile
from concourse.bass_utils import run_bass_kernel_spmd

NCORES = 8
DIM = 200
R = 16
RR = R * R  # 256
NTAB = DIM * DIM  # 40000
HALF = 32768
N = 500_000
P = 128
CHUNK = 1024  # rows per dma_gather (2048 fails on HW)
SUB = CHUNK // P  # 16 sub-tiles of 128 rows per chunk


def _tables(core0, core1, core2, core3, lam0, lam1, lam2, lam3):
    A0 = (core0 * lam0[None, None, :]).astype(np.float32)
    A1 = (core1 * lam1[None, None, :]).astype(np.float32)
    A2 = (core2 * lam2[None, None, :]).astype(np.float32)
    A3 = (core3 * lam3[None, None, :]).astype(np.float32)
    # T1[(j0,j1)][x,z] = sum_y A0[j0,x,y] A1[j1,y,z]
    M1 = A0.reshape(DIM * R, R) @ np.ascontiguousarray(A1.transpose(1, 0, 2)).reshape(
        R, DIM * R
    )  # [(j0 x), (j1 z)]
    T1 = np.ascontiguousarray(
        M1.reshape(DIM, R, DIM, R).transpose(0, 2, 1, 3)
    ).reshape(NTAB, RR)
    # T2[(j2,j3)][x,z] = sum_w A2[j2,z,w] A3[j3,w,x]
    M2 = A2.reshape(DIM * R, R) @ np.ascontiguousarray(A3.transpose(1, 0, 2)).reshape(
        R, DIM * R
    )  # [(j2 z), (j3 x)]
    T2 = np.ascontiguousarray(
        M2.reshape(DIM, R, DIM, R).transpose(0, 2, 3, 1)
    ).reshape(NTAB, RR)
    return T1.astype(np.float16), T2.astype(np.float16)


def _wrap16(a):
    """Pack a [n*128] idx list into dma_gather layout [128, n*8]:
    idx i -> [i % 16, i // 16], replicated over the 8 partition groups."""
    w = np.ascontiguousarray(a.reshape(-1, 16).T)  # [16, len/16]
    return np.ascontiguousarray(np.tile(w, (8, 1)))  # [128, len/16]


def build_program(flavors):
    """flavors: tuple of (p_hi, q_hi) per chunk — static table-half selection."""
    nch = len(flavors)
    nc = bacc.Bacc("TRN2", target_bir_lowering=False, num_swdge_queues=4)
    f16 = mybir.dt.float16
    f32 = mybir.dt.float32
    i16 = mybir.dt.int16

    t1 = nc.dram_tensor("t1", [NTAB, RR], f16, kind="ExternalInput")
    t2 = nc.dram_tensor("t2", [NTAB, RR], f16, kind="ExternalInput")
    # per chunk: CHUNK idxs wrapped to [128, CHUNK//16]
    pidx = nc.dram_tensor("pidx", [P, nch * SUB * 8], i16, kind="ExternalInput")
    qidx = nc.dram_tensor("qidx", [P, nch * SUB * 8], i16, kind="ExternalInput")
    out = nc.dram_tensor("out", [P, nch * SUB], f32, kind="ExternalOutput")

    IW = SUB * 8  # idx words per chunk per partition (CHUNK/16)

    with tile.TileContext(nc) as tc:
        with (
            tc.tile_pool(name="idx", bufs=8) as idx_pool,
            tc.tile_pool(name="g1", bufs=6) as g1_pool,
            tc.tile_pool(name="g2", bufs=6) as g2_pool,
            tc.tile_pool(name="prod", bufs=4) as prod_pool,
            tc.tile_pool(name="res", bufs=1) as res_pool,
        ):
            out_sb = res_pool.tile([P, nch * SUB], f32)

            for c, (p_hi, q_hi) in enumerate(flavors):
                src1 = t1[HALF:NTAB, :] if p_hi else t1[0:HALF, :]
                src2 = t2[HALF:NTAB, :] if q_hi else t2[0:HALF, :]
                p_sb = idx_pool.tile([P, IW], i16, tag="p")
                q_sb = idx_pool.tile([P, IW], i16, tag="q")
                nc.sync.dma_start(out=p_sb[:], in_=pidx[:, c * IW : (c + 1) * IW])
                nc.sync.dma_start(out=q_sb[:], in_=qidx[:, c * IW : (c + 1) * IW])
                g1 = g1_pool.tile([P, SUB, RR], f16)
                g2 = g2_pool.tile([P, SUB, RR], f16)
                nc.gpsimd.dma_gather(
                    g1[:], src1, p_sb[:], CHUNK, CHUNK, RR,
                    queue_num=(2 * c) % 4,
                )
                nc.gpsimd.dma_gather(
                    g2[:], src2, q_sb[:], CHUNK, CHUNK, RR,
                    queue_num=(2 * c + 1) % 4,
                )
                prod = prod_pool.tile([P, SUB, RR], f16)
                nc.vector.tensor_tensor(
                    out=prod[:], in0=g1[:], in1=g2[:], op=mybir.AluOpType.mult
                )
                nc.vector.tensor_reduce(
                    out=out_sb[:, c * SUB : (c + 1) * SUB],
                    in_=prod[:],
                    axis=mybir.AxisListType.X,
                    op=mybir.AluOpType.add,
                )
            nc.sync.dma_start(out=out[:], in_=out_sb[:])
    nc.compile()
    return nc


_PROG_CACHE = {}


def _get_program(flavors):
    key = tuple(flavors)
    if key not in _PROG_CACHE:
        _PROG_CACHE[key] = build_program(key)
    return _PROG_CACHE[key]


def plan(index):
    """Bucket rows per core, build a shared chunk grid + per-core idx arrays."""
    idx = np.asarray(index).astype(np.int64)
    p_all = (idx[:, 0] * DIM + idx[:, 1]).astype(np.int32)
    q_all = (idx[:, 2] * DIM + idx[:, 3]).astype(np.int32)
    rows = N // NCORES
    per_core = []
    counts = np.zeros((NCORES, 4), np.int64)
    for c in range(NCORES):
        sl = slice(c * rows, (c + 1) * rows)
        p, q = p_all[sl], q_all[sl]
        b = (p >= HALF) * 2 + (q >= HALF)
        ids = [np.where(b == bb)[0] for bb in range(4)]
        counts[c] = [len(x) for x in ids]
        per_core.append((p, q, ids))
    nch_b = [max(1, -(-int(counts[:, bb].max()) // CHUNK)) for bb in range(4)]
    flavors = []
    for bb in range(4):
        flavors += [(bb >> 1 & 1, bb & 1)] * nch_b[bb]
    return per_core, nch_b, tuple(flavors)


def make_in_maps(per_core, nch_b, T1, T2):
    in_maps, metas = [], []
    for p, q, ids in per_core:
        p16s, q16s, meta = [], [], []
        for bb in range(4):
            cap = nch_b[bb] * CHUNK
            sel = ids[bb]
            pd = np.full(cap, HALF if (bb >> 1 & 1) else 0, np.int32)
            qd = np.full(cap, HALF if (bb & 1) else 0, np.int32)
            pd[: len(sel)] = p[sel]
            qd[: len(sel)] = q[sel]
            p16s.append((pd - (HALF if (bb >> 1 & 1) else 0)).astype(np.int16))
            q16s.append((qd - (HALF if (bb & 1) else 0)).astype(np.int16))
            meta.append(sel)
        p16 = np.concatenate(p16s)
        q16 = np.concatenate(q16s)
        # wrap each chunk independently and lay side by side on the free dim
        nch = len(p16) // CHUNK
        pw = np.concatenate(
            [_wrap16(p16[i * CHUNK : (i + 1) * CHUNK]) for i in range(nch)], axis=1
        )
        qw = np.concatenate(
            [_wrap16(q16[i * CHUNK : (i + 1) * CHUNK]) for i in range(nch)], axis=1
        )
        in_maps.append({"t1": T1, "t2": T2, "pidx": pw, "qidx": qw})
        metas.append(meta)
    return in_maps, metas


def unpack(results, metas, nch_b):
    rows = N // NCORES
    outs = []
    for c in range(NCORES):
        o = results[c]["out"]  # [128, nch*SUB]; slot s -> [s%128, s//128]
        flat = np.asarray(o).T.reshape(-1)  # slot-ordered
        full = np.empty(rows, np.float32)
        off = 0
        for bb in range(4):
            cap = nch_b[bb] * CHUNK
            sel = metas[c][bb]
            full[sel] = flat[off : off + len(sel)]
            off += cap
        outs.append(full)
    return np.concatenate(outs)


def kernel(index, core0, core1, core2, core3, lam0, lam1, lam2, lam3, _trace=False):
    T1, T2 = _tables(
        np.asarray(core0), np.asarray(core1), np.asarray(core2), np.asarray(core3),
        np.asarray(lam0), np.asarray(lam1), np.asarray(lam2), np.asarray(lam3),
    )
    per_core, nch_b, flavors = plan(index)
    nc = _get_program(flavors)
    in_maps, metas = make_in_maps(per_core, nch_b, T1, T2)
    res = run_bass_kernel_spmd(
        nc, in_maps, core_ids=list(range(NCORES)), trace=_trace
    )
    full = unpack(res.results, metas, nch_b).astype(np.float32)
    if _trace:
        return full, res
    return full


# revision 11
# speedup vs baseline: 1.1008x; 1.0216x over previous
"""Bass/Trainium2 kernel for the BayesTensorRing embedding-lookup problem.

out[i] = trace( prod_d (core_d[idx_d] * diag(lam_d)) ),  N=500k rows, 8 cores.

Strategy:
  * Host (one-time weight repacking, independent of the index stream):
    fold each lambda into its core, then build two pair-product tables
       T1[(j0,j1)][x,z] = sum_y A0[j0,x,y] A1[j1,y,z]     (A0A1)
       T2[(j2,j3)][x,z] = sum_w A2[j2,z,w] A3[j3,w,x]     ((A2A3)^T layout)
    each [40000, 256] fp16.  Then out[i] = <T1[p_i], T2[q_i]> elementwise dot,
    p = i0*200+i1, q = i2*200+i3.
  * Device (all per-row work): shard N over 8 NeuronCores. Rows are bucketed
    by (p>=32768, q>=32768) so each bulk dma_gather chunk addresses one table
    half with int16 indices. Per 2048-row chunk: two dma_gathers (512B rows)
    + one DVE multiply + one DVE reduce -> 2048 fp32 outputs.
"""

import os
import sys

import numpy as np

sys.path.insert(0, "/opt/trn_rl_repo")

from concourse import bacc, bass, mybir
import concourse.tile as tile
from concourse.bass_utils import run_bass_kernel_spmd

NCORES = 8
DIM = 200
R = 16
RR = R * R  # 256
NTAB = DIM * DIM  # 40000
HALF = 32768
N = 500_000
P = 128
CHUNK = 1024  # rows per dma_gather (2048 fails on HW)
SUB = CHUNK // P  # 16 sub-tiles of 128 rows per chunk


def _tables(core0, core1, core2, core3, lam0, lam1, lam2, lam3):
    A0 = (core0 * lam0[None, None, :]).astype(np.float32)
    A1 = (core1 * lam1[None, None, :]).astype(np.float32)
    A2 = (core2 * lam2[None, None, :]).astype(np.float32)
    A3 = (core3 * lam3[None, None, :]).astype(np.float32)
    # T1[(j0,j1)][x,z] = sum_y A0[j0,x,y] A1[j1,y,z]
    M1 = A0.reshape(DIM * R, R) @ np.ascontiguousarray(A1.transpose(1, 0, 2)).reshape(
        R, DIM * R
    )  # [(j0 x), (j1 z)]
    T1 = np.ascontiguousarray(
        M1.reshape(DIM, R, DIM, R).transpose(0, 2, 1, 3)
    ).reshape(NTAB, RR)
    # T2[(j2,j3)][x,z] = sum_w A2[j2,z,w] A3[j3,w,x]
    M2 = A2.reshape(DIM * R, R) @ np.ascontiguousarray(A3.transpose(1, 0, 2)).reshape(
        R, DIM * R
    )  # [(j2 z), (j3 x)]
    T2 = np.ascontiguousarray(
        M2.reshape(DIM, R, DIM, R).transpose(0, 2, 3, 1)
    ).reshape(NTAB, RR)
    return T1.astype(np.float16), T2.astype(np.float16)


def _wrap16(a):
    """Pack a [n*128] idx list into dma_gather layout [128, n*8]:
    idx i -> [i % 16, i // 16], replicated over the 8 partition groups."""
    w = np.ascontiguousarray(a.reshape(-1, 16).T)  # [16, len/16]
    return np.ascontiguousarray(np.tile(w, (8, 1)))  # [128, len/16]


def build_program(flavors):
    """flavors: tuple of (p_hi, q_hi) per chunk — static table-half selection."""
    nch = len(flavors)
    nc = bacc.Bacc("TRN2", target_bir_lowering=False, num_swdge_queues=4)
    f16 = mybir.dt.float16
    f32 = mybir.dt.float32
    i16 = mybir.dt.int16

    t1 = nc.dram_tensor("t1", [NTAB, RR], f16, kind="ExternalInput")
    t2 = nc.dram_tensor("t2", [NTAB, RR], f16, kind="ExternalInput")
    # per chunk: CHUNK idxs wrapped to [128, CHUNK//16]
    pidx = nc.dram_tensor("pidx", [P, nch * SUB * 8], i16, kind="ExternalInput")
    qidx = nc.dram_tensor("qidx", [P, nch * SUB * 8], i16, kind="ExternalInput")
    out = nc.dram_tensor("out", [P, nch * SUB], f32, kind="ExternalOutput")

    IW = SUB * 8  # idx words per chunk per partition (CHUNK/16)

    with tile.TileContext(nc) as tc:
        with (
            tc.tile_pool(name="idx", bufs=8) as idx_pool,
            tc.tile_pool(name="g1", bufs=6) as g1_pool,
            tc.tile_pool(name="g2", bufs=6) as g2_pool,
            tc.tile_pool(name="prod", bufs=4) as prod_pool,
            tc.tile_pool(name="res", bufs=1) as res_pool,
        ):
            out_sb = res_pool.tile([P, nch * SUB], f32)

            for c, (p_hi, q_hi) in enumerate(flavors):
                src1 = t1[HALF:NTAB, :] if p_hi else t1[0:HALF, :]
                src2 = t2[HALF:NTAB, :] if q_hi else t2[0:HALF, :]
                p_sb = idx_pool.tile([P, IW], i16, tag="p")
                q_sb = idx_pool.tile([P, IW], i16, tag="q")
                nc.sync.dma_start(out=p_sb[:], in_=pidx[:, c * IW : (c + 1) * IW])
                nc.sync.dma_start(out=q_sb[:], in_=qidx[:, c * IW : (c + 1) * IW])
                g1 = g1_pool.tile([P, SUB, RR], f16)
                g2 = g2_pool.tile([P, SUB, RR], f16)
                nc.gpsimd.dma_gather(
                    g1[:], src1, p_sb[:], CHUNK, CHUNK, RR,
                    queue_num=(2 * c) % 4,
                )
                nc.gpsimd.dma_gather(
                    g2[:], src2, q_sb[:], CHUNK, CHUNK, RR,
                    queue_num=(2 * c + 1) % 4,
                )
                prod = prod_pool.tile([P, SUB, RR], f16)
                nc.vector.tensor_tensor(
                    out=prod[:], in0=g1[:], in1=g2[:], op=mybir.AluOpType.mult
                )
                nc.vector.tensor_reduce(
                    out=out_sb[:, c * SUB : (c + 1) * SUB],
                    in_=prod[:],
                    axis=mybir.AxisListType.X,
                    op=mybir.AluOpType.add,
                )
            nc.sync.dma_start(out=out[:], in_=out_sb[:])
    nc.compile()
    return nc


_PROG_CACHE = {}


def _get_program(flavors):
    key = tuple(flavors)
    if key not in _PROG_CACHE:
        _PROG_CACHE[key] = build_program(key)
    return _PROG_CACHE[key]


def plan(index):
    """Bucket rows per core, build a shared chunk grid + per-core idx arrays."""
    idx = np.asarray(index).astype(np.int64)
    p_all = (idx[:, 0] * DIM + idx[:, 1]).astype(np.int32)
    q_all = (idx[:, 2] * DIM + idx[:, 3]).astype(np.int32)
    rows = N // NCORES
    per_core = []
    counts = np.zeros((NCORES, 4), np.int64)
    for c in range(NCORES):
        sl = slice(c * rows, (c + 1) * rows)
        p, q = p_all[sl], q_all[sl]
        b = (p >= HALF) * 2 + (q >= HALF)
        # sort each bucket by p: sequential-ish T1 reads are HBM-friendlier
        ids = []
        for bb in range(4):
            sel = np.where(b == bb)[0]
            ids.append(sel[np.argsort(p[sel], kind="stable")])
        counts[c] = [len(x) for x in ids]
        per_core.append((p, q, ids))
    nch_b = [max(1, -(-int(counts[:, bb].max()) // CHUNK)) for bb in range(4)]
    flavors = []
    for bb in range(4):
        flavors += [(bb >> 1 & 1, bb & 1)] * nch_b[bb]
    return per_core, nch_b, tuple(flavors)


def make_in_maps(per_core, nch_b, T1, T2):
    in_maps, metas = [], []
    for p, q, ids in per_core:
        p16s, q16s, meta = [], [], []
        for bb in range(4):
            cap = nch_b[bb] * CHUNK
            sel = ids[bb]
            pd = np.full(cap, HALF if (bb >> 1 & 1) else 0, np.int32)
            qd = np.full(cap, HALF if (bb & 1) else 0, np.int32)
            pd[: len(sel)] = p[sel]
            qd[: len(sel)] = q[sel]
            p16s.append((pd - (HALF if (bb >> 1 & 1) else 0)).astype(np.int16))
            q16s.append((qd - (HALF if (bb & 1) else 0)).astype(np.int16))
            meta.append(sel)
        p16 = np.concatenate(p16s)
        q16 = np.concatenate(q16s)
        # wrap each chunk independently and lay side by side on the free dim
        nch = len(p16) // CHUNK
        pw = np.concatenate(
            [_wrap16(p16[i * CHUNK : (i + 1) * CHUNK]) for i in range(nch)], axis=1
        )
        qw = np.concatenate(
            [_wrap16(q16[i * CHUNK : (i + 1) * CHUNK]) for i in range(nch)], axis=1
        )
        in_maps.append({"t1": T1, "t2": T2, "pidx": pw, "qidx": qw})
        metas.append(meta)
    return in_maps, metas


def unpack(results, metas, nch_b):
    rows = N // NCORES
    outs = []
    for c in range(NCORES):
        o = results[c]["out"]  # [128, nch*SUB]; slot s -> [s%128, s//128]
        flat = np.asarray(o).T.reshape(-1)  # slot-ordered
        full = np.empty(rows, np.float32)
        off = 0
        for bb in range(4):
            cap = nch_b[bb] * CHUNK
            sel = metas[c][bb]
            full[sel] = flat[off : off + len(sel)]
            off += cap
        outs.append(full)
    return np.concatenate(outs)


def kernel(index, core0, core1, core2, core3, lam0, lam1, lam2, lam3, _trace=False):
    T1, T2 = _tables(
        np.asarray(core0), np.asarray(core1), np.asarray(core2), np.asarray(core3),
        np.asarray(lam0), np.asarray(lam1), np.asarray(lam2), np.asarray(lam3),
    )
    per_core, nch_b, flavors = plan(index)
    nc = _get_program(flavors)
    in_maps, metas = make_in_maps(per_core, nch_b, T1, T2)
    res = run_bass_kernel_spmd(
        nc, in_maps, core_ids=list(range(NCORES)), trace=_trace
    )
    full = unpack(res.results, metas, nch_b).astype(np.float32)
    if _trace:
        return full, res
    return full


# revision 13
# speedup vs baseline: 1.1068x; 1.0055x over previous
"""Bass/Trainium2 kernel for the BayesTensorRing embedding-lookup problem.

out[i] = trace( prod_d (core_d[idx_d] * diag(lam_d)) ),  N=500k rows, 8 cores.

Strategy:
  * Host (one-time weight repacking, independent of the index stream):
    fold each lambda into its core, then build two pair-product tables
       T1[(j0,j1)][x,z] = sum_y A0[j0,x,y] A1[j1,y,z]     (A0A1)
       T2[(j2,j3)][x,z] = sum_w A2[j2,z,w] A3[j3,w,x]     ((A2A3)^T layout)
    each [40000, 256] fp16.  Then out[i] = <T1[p_i], T2[q_i]> elementwise dot,
    p = i0*200+i1, q = i2*200+i3.
  * Device (all per-row work): shard N over 8 NeuronCores. Rows are bucketed
    by (p>=32768, q>=32768) so each bulk dma_gather chunk addresses one table
    half with int16 indices. Per 2048-row chunk: two dma_gathers (512B rows)
    + one DVE multiply + one DVE reduce -> 2048 fp32 outputs.
"""

import sys

import numpy as np

sys.path.insert(0, "/opt/trn_rl_repo")

from concourse import bacc, mybir
import concourse.tile as tile
from concourse.bass_utils import run_bass_kernel_spmd

NCORES = 8
DIM = 200
R = 16
RR = R * R  # 256
NTAB = DIM * DIM  # 40000
HALF = 32768
N = 500_000
P = 128
CHUNK = 1024  # rows per dma_gather (2048 fails on HW)
SUB = CHUNK // P  # 16 sub-tiles of 128 rows per chunk


def _tables(core0, core1, core2, core3, lam0, lam1, lam2, lam3):
    A0 = (core0 * lam0[None, None, :]).astype(np.float32)
    A1 = (core1 * lam1[None, None, :]).astype(np.float32)
    A2 = (core2 * lam2[None, None, :]).astype(np.float32)
    A3 = (core3 * lam3[None, None, :]).astype(np.float32)
    # T1[(j0,j1)][x,z] = sum_y A0[j0,x,y] A1[j1,y,z]
    M1 = A0.reshape(DIM * R, R) @ np.ascontiguousarray(A1.transpose(1, 0, 2)).reshape(
        R, DIM * R
    )  # [(j0 x), (j1 z)]
    T1 = np.ascontiguousarray(
        M1.reshape(DIM, R, DIM, R).transpose(0, 2, 1, 3)
    ).reshape(NTAB, RR)
    # T2[(j2,j3)][x,z] = sum_w A2[j2,z,w] A3[j3,w,x]
    M2 = A2.reshape(DIM * R, R) @ np.ascontiguousarray(A3.transpose(1, 0, 2)).reshape(
        R, DIM * R
    )  # [(j2 z), (j3 x)]
    T2 = np.ascontiguousarray(
        M2.reshape(DIM, R, DIM, R).transpose(0, 2, 3, 1)
    ).reshape(NTAB, RR)
    return T1.astype(np.float16), T2.astype(np.float16)


def _wrap16(a):
    """Pack a [n*128] idx list into dma_gather layout [128, n*8]:
    idx i -> [i % 16, i // 16], replicated over the 8 partition groups."""
    w = np.ascontiguousarray(a.reshape(-1, 16).T)  # [16, len/16]
    return np.ascontiguousarray(np.tile(w, (8, 1)))  # [128, len/16]


def build_program(flavors):
    """flavors: tuple of (p_hi, q_hi) per chunk — static table-half selection."""
    nch = len(flavors)
    nc = bacc.Bacc("TRN2", target_bir_lowering=False, num_swdge_queues=4)
    f16 = mybir.dt.float16
    f32 = mybir.dt.float32
    i16 = mybir.dt.int16

    t1 = nc.dram_tensor("t1", [NTAB, RR], f16, kind="ExternalInput")
    t2 = nc.dram_tensor("t2", [NTAB, RR], f16, kind="ExternalInput")
    # per chunk: CHUNK idxs wrapped to [128, CHUNK//16]
    pidx = nc.dram_tensor("pidx", [P, nch * SUB * 8], i16, kind="ExternalInput")
    qidx = nc.dram_tensor("qidx", [P, nch * SUB * 8], i16, kind="ExternalInput")
    out = nc.dram_tensor("out", [P, nch * SUB], f32, kind="ExternalOutput")

    IW = SUB * 8  # idx words per chunk per partition (CHUNK/16)

    with tile.TileContext(nc) as tc:
        with (
            tc.tile_pool(name="idx", bufs=8) as idx_pool,
            tc.tile_pool(name="g1", bufs=6) as g1_pool,
            tc.tile_pool(name="g2", bufs=6) as g2_pool,
            tc.tile_pool(name="prod", bufs=4) as prod_pool,
            tc.tile_pool(name="res", bufs=1) as res_pool,
        ):
            out_sb = res_pool.tile([P, nch * SUB], f32)

            for c, (p_hi, q_hi) in enumerate(flavors):
                src1 = t1[HALF:NTAB, :] if p_hi else t1[0:HALF, :]
                src2 = t2[HALF:NTAB, :] if q_hi else t2[0:HALF, :]
                p_sb = idx_pool.tile([P, IW], i16, tag="p")
                q_sb = idx_pool.tile([P, IW], i16, tag="q")
                nc.sync.dma_start(out=p_sb[:], in_=pidx[:, c * IW : (c + 1) * IW])
                nc.sync.dma_start(out=q_sb[:], in_=qidx[:, c * IW : (c + 1) * IW])
                g1 = g1_pool.tile([P, SUB, RR], f16)
                g2 = g2_pool.tile([P, SUB, RR], f16)
                nc.gpsimd.dma_gather(
                    g1[:], src1, p_sb[:], CHUNK, CHUNK, RR,
                    queue_num=c % 4,
                )
                nc.gpsimd.dma_gather(
                    g2[:], src2, q_sb[:], CHUNK, CHUNK, RR,
                    queue_num=(c + 2) % 4,
                )
                prod = prod_pool.tile([P, SUB, RR], f16)
                nc.vector.tensor_tensor(
                    out=prod[:], in0=g1[:], in1=g2[:], op=mybir.AluOpType.mult
                )
                nc.vector.tensor_reduce(
                    out=out_sb[:, c * SUB : (c + 1) * SUB],
                    in_=prod[:],
                    axis=mybir.AxisListType.X,
                    op=mybir.AluOpType.add,
                )
            nc.sync.dma_start(out=out[:], in_=out_sb[:])
    nc.compile()
    return nc


_PROG_CACHE = {}


def _get_program(flavors):
    key = tuple(flavors)
    if key not in _PROG_CACHE:
        _PROG_CACHE[key] = build_program(key)
    return _PROG_CACHE[key]


def plan(index):
    """Bucket rows per core, build a shared chunk grid + per-core idx arrays."""
    idx = np.asarray(index).astype(np.int64)
    p_all = (idx[:, 0] * DIM + idx[:, 1]).astype(np.int32)
    q_all = (idx[:, 2] * DIM + idx[:, 3]).astype(np.int32)
    rows = N // NCORES
    per_core = []
    counts = np.zeros((NCORES, 4), np.int64)
    for c in range(NCORES):
        sl = slice(c * rows, (c + 1) * rows)
        p, q = p_all[sl], q_all[sl]
        b = (p >= HALF) * 2 + (q >= HALF)
        # sort each bucket by p: sequential-ish T1 reads are HBM-friendlier
        ids = []
        for bb in range(4):
            sel = np.where(b == bb)[0]
            ids.append(sel[np.argsort(p[sel], kind="stable")])
        counts[c] = [len(x) for x in ids]
        per_core.append((p, q, ids))
    nch_b = [max(1, -(-int(counts[:, bb].max()) // CHUNK)) for bb in range(4)]
    flavors = []
    for bb in range(4):
        flavors += [(bb >> 1 & 1, bb & 1)] * nch_b[bb]
    return per_core, nch_b, tuple(flavors)


def make_in_maps(per_core, nch_b, T1, T2):
    in_maps, metas = [], []
    for p, q, ids in per_core:
        p16s, q16s, meta = [], [], []
        for bb in range(4):
            cap = nch_b[bb] * CHUNK
            sel = ids[bb]
            pd = np.full(cap, HALF if (bb >> 1 & 1) else 0, np.int32)
            qd = np.full(cap, HALF if (bb & 1) else 0, np.int32)
            pd[: len(sel)] = p[sel]
            qd[: len(sel)] = q[sel]
            p16s.append((pd - (HALF if (bb >> 1 & 1) else 0)).astype(np.int16))
            q16s.append((qd - (HALF if (bb & 1) else 0)).astype(np.int16))
            meta.append(sel)
        p16 = np.concatenate(p16s)
        q16 = np.concatenate(q16s)
        # wrap each chunk independently and lay side by side on the free dim
        nch = len(p16) // CHUNK
        pw = np.concatenate(
            [_wrap16(p16[i * CHUNK : (i + 1) * CHUNK]) for i in range(nch)], axis=1
        )
        qw = np.concatenate(
            [_wrap16(q16[i * CHUNK : (i + 1) * CHUNK]) for i in range(nch)], axis=1
        )
        in_maps.append({"t1": T1, "t2": T2, "pidx": pw, "qidx": qw})
        metas.append(meta)
    return in_maps, metas


def unpack(results, metas, nch_b):
    rows = N // NCORES
    outs = []
    for c in range(NCORES):
        o = results[c]["out"]  # [128, nch*SUB]; slot s -> [s%128, s//128]
        flat = np.asarray(o).T.reshape(-1)  # slot-ordered
        full = np.empty(rows, np.float32)
        off = 0
        for bb in range(4):
            cap = nch_b[bb] * CHUNK
            sel = metas[c][bb]
            full[sel] = flat[off : off + len(sel)]
            off += cap
        outs.append(full)
    return np.concatenate(outs)


def kernel(index, core0, core1, core2, core3, lam0, lam1, lam2, lam3, _trace=False):
    T1, T2 = _tables(
        np.asarray(core0), np.asarray(core1), np.asarray(core2), np.asarray(core3),
        np.asarray(lam0), np.asarray(lam1), np.asarray(lam2), np.asarray(lam3),
    )
    per_core, nch_b, flavors = plan(index)
    nc = _get_program(flavors)
    in_maps, metas = make_in_maps(per_core, nch_b, T1, T2)
    res = run_bass_kernel_spmd(
        nc, in_maps, core_ids=list(range(NCORES)), trace=_trace
    )
    full = unpack(res.results, metas, nch_b).astype(np.float32)
    if _trace:
        return full, res
    return full
